# revision 34
# baseline (speedup 1.0000x reference)
"""Trainium2 Bass kernel for nn_LNon_37460704756094 (embedding_lookup).

Math (reference):
    d   = (data - mean(data)) / std(data, ddof=1) * scalei
    s   = sigmoid(d); t = tanh(d)
    theta = interp(theta_lut, s * 119)   # theta_lut = linspace(-pi, pi, 120)
    velo  = interp(velo_lut, |t| * 119)  # velo_lut  = linspace(0, 3, 120)
    val = d * exp(velo * sin(theta)) + velo * cos(theta)
    out = (val - mean(val)) / std(val, ddof=1) * scaleo

Affine LUTs + sigmoid(d) = (1 + tanh(d/2))/2 give
    theta = th_mid + th_half * tanh(d/2);  velo = v_slope * |tanh(d)|

Per core (shard [128, 32768] f32):
  A (~55-62us, DMA-bound): 16 chunks of 2048 cols stream through a 5-deep
      fp32 staging ring recycled on the scalar Copy alone (recycling on a
      vector op stalls the DMA queues and starves the cc-stream warmup).
      Copy+accum -> resident fp16 x16 and sum(x); sum(x^2) via a vector
      stt reading x16 (product dumped into dead pb).
  AG1: one dummy AllGather issued as the first gpsimd instruction absorbs
      the cc-stream bring-up (begin is launch+60..95us regardless of
      trigger time; an AllReduce here stalls ~265us). The real AllGather
      ([P,2] partials) then runs semi-warm; PE ones-matmul broadcasts the
      partition reduction; an 8-op chain (AP-scalar tensor_scalar forms)
      makes a = scalei/std, b = -mean*a.
  Mid (~161us, ACT-bound; geometrically tapered quarters
      10240/8192/6144/4096/4096 so each quarter's DVE chain drains inside
      the next quarter's activation window -- ratio >= ~0.7 is the
      stall-free condition):
      t2 = tanh((ax+b)/2) -> pb | T = tanh(ax+b) -> Tb (shared table)
      u = a*x+b in place on dead x16 (2x tensor_scalar)
      |T|: fp16 sign-bit clear via uint16 bitwise_and (2x) -- stt is 1x
      cos -> quarter-local qb ping-pong, sin -> pb in place (one table)
      q = qb*|T|, p = pb*|T| (4x fp16 tensor_tensor)
      exp in place on pb | e' = u*e (4x) | val = v_slope*q + e' (stt,
      accum sum) | sum(val^2) on the idle PE: per-tile gram matmuls
      accumulate into one PSUM bank; the diagonal is extracted once at
      the end with an affine_select identity + reduce.
      The tile framework reschedules per-engine streams itself (it
      batches activation tables); no manual lag staging.
  AG2 (kept warm by a mid-phase dummy AllGather that data-depends on the
      a,b chain), then D (~60us, store-bound): out = a2*val + b2
      alternating scalar Identity / vector tensor_scalar into a 4-deep
      fp32 ring, stores alternating sync/scalar DMA queues.

fp16 intermediates keep rel err ~1.6e-3 (gate 2e-2). Engine clock and
collective timing vary ~20% run-to-run; ~346-392us raw, ~350 typical.
"""

import math

import numpy as np

import concourse.bacc as bacc
import concourse.bass as bass
import concourse.mybir as mybir
import concourse.tile as tile
from concourse.bass_utils import run_bass_kernel_spmd

N_CORES = 8
P = 128
B_FULL, C, H, W = 32, 64, 128, 128
PER_CORE = B_FULL // N_CORES * C * H * W          # 4,194,304
FREE = PER_CORE // P                              # 32,768
N_TOTAL = B_FULL * C * H * W                      # 33,554,432

CA = 2048                                         # phase-A chunk
NA = FREE // CA                                   # 16
QWS = (10240, 8192, 6144, 4096, 4096)             # quarter widths (sum FREE)
QMAX = max(QWS)
NQ = len(QWS)
CT = 2048                                         # p/q-mult chunk
CC = 2048                                         # exp/val chunk
NC = FREE // CC                                   # 16
CD = 2048                                         # store chunk
ND = FREE // CD                                   # 16

AF = mybir.ActivationFunctionType
ALU = mybir.AluOpType
AX = mybir.AxisListType
F32 = mybir.dt.float32
F16 = mybir.dt.float16

LAST_RESULT = None  # BassKernelResults of the most recent run (for test.py)

_KERNEL_CACHE = {}


def _build(consts, sim_mode=False):
    """consts = (th_mid, th_half, v_slope)."""
    th_mid, th_half, v_slope = consts
    halfpi = math.pi / 2.0

    nc = bacc.Bacc(None, num_devices=N_CORES)

    data_in = nc.dram_tensor("data", [P, FREE], F32, kind="ExternalInput")
    scal_in = nc.dram_tensor("scal", [P, 2], F32, kind="ExternalInput")
    out_dram = nc.dram_tensor("out", [P, FREE], F32, kind="ExternalOutput")

    groups = [list(range(N_CORES))]

    def all_gather(cc_in, cc_out):
        if sim_mode:
            for k in range(N_CORES):
                nc.gpsimd.dma_start(cc_out[k], cc_in[:])
        else:
            nc.gpsimd.collective_compute(
                "AllGather", ALU.bypass, replica_groups=groups,
                ins=[cc_in.opt()], outs=[cc_out.opt()],
            )

    NT = float(N_TOTAL) if not sim_mode else float(PER_CORE)
    c_m2 = -1.0 / (NT * (NT - 1.0))
    c_v = 1.0 / (NT - 1.0)
    c_mn = -1.0 / NT

    with tile.TileContext(nc) as tc:
        with (
            tc.tile_pool(name="keep", bufs=1) as keep,
            tc.tile_pool(name="psum", bufs=1, space="PSUM") as psumpool,
            tc.tile_pool(name="dram", bufs=1, space="DRAM") as dram,
        ):
            # ------- persistent SBUF -------
            x16 = keep.tile([P, FREE], F16, name="x16", tag="x16")
            pb = keep.tile([P, FREE], F16, name="pb", tag="pb")
            # qb holds cos*|T| only within its own quarter: two ping-pong
            # quarter-sized buffers instead of a full-FREE tile (saves 32KB)
            qbuf = [
                keep.tile(
                    [P, max(QWS[i::2])], F16, name=f"qb{i}", tag=f"qb{i}"
                )
                for i in range(2)
            ]
            statA = keep.tile([P, 2 * NA], F32, name="statA", tag="statA")
            statC = keep.tile([P, 2 * NC], F32, name="statC", tag="statC")
            sm = keep.tile([P, 32], F32, name="sm", tag="sm")
            stA = keep.tile([P, 2], F32, name="stA", tag="stA")
            stB = keep.tile([P, 2], F32, name="stB", tag="stB")
            sAg = keep.tile([P, 2 * N_CORES], F32, name="sAg", tag="sAg")
            sBg = keep.tile([P, 2 * N_CORES], F32, name="sBg", tag="sBg")
            scal_all = keep.tile([P, 2], F32, name="scal_all", tag="scal_all")
            ones = keep.tile([P, P], F32, name="ones", tag="ones")
            psumA = psumpool.tile([P, 2], F32, name="psumA", tag="psumA")
            psumB = psumpool.tile([P, 2], F32, name="psumB", tag="psumB")
            psumV = psumpool.tile([P, P], F32, name="psumV", tag="psumV")
            iden = keep.tile([P, P], F32, name="iden", tag="iden")
            vdiag = keep.tile([P, P], F32, name="vdiag", tag="vdiag")

            cc_w_in = dram.tile([P, 2], F32, name="cc_w_in", tag="cc_w_in")
            cc_w_out = dram.tile([N_CORES, P, 2], F32, name="cc_w_out", tag="cc_w_out")
            cc_a_in = dram.tile([P, 2], F32, name="cc_a_in", tag="cc_a_in")
            cc_a_out = dram.tile([N_CORES, P, 2], F32, name="cc_a_out", tag="cc_a_out")
            cc_b_in = dram.tile([P, 2], F32, name="cc_b_in", tag="cc_b_in")
            cc_b_out = dram.tile([N_CORES, P, 2], F32, name="cc_b_out", tag="cc_b_out")

            # One dummy AllGather as the very first gpsimd instruction: the
            # first collective's begin+duration are highly variable (launch
            # +60..95us); letting the dummy absorb that keeps the real AG1
            # at a stable ~77+12us. (An AllReduce here stalls ~265us.)
            all_gather(cc_w_in, cc_w_out)

            nc.sync.dma_start(scal_all[:], scal_in[:])
            nc.vector.memset(ones[:], 1.0)
            # identity matrix: keep ones where (p - j) == 0
            nc.gpsimd.affine_select(
                iden[:], ones[:], pattern=[[-1, P]], base=0,
                channel_multiplier=1, compare_op=ALU.is_equal, fill=0.0,
            )
            # Sin bias constants as plain sm columns (no const-AP barrier)
            nc.vector.memset(sm[:, 12:13], th_mid)
            nc.vector.memset(sm[:, 13:14], th_mid + halfpi)

            # ---------------- Phase A: load + convert + input stats ------
            with tc.tile_pool(name="pxa", bufs=1) as pxa:
                xa = [
                    pxa.tile([P, CA], F32, name=f"xa{i}", tag=f"xa{i}")
                    for i in range(5)
                ]
                for j in range(NA):
                    sl = slice(j * CA, (j + 1) * CA)
                    x_ = xa[j % 5]
                    nc.sync.dma_start(x_[:], data_in[:, sl])
                    # fp32 -> fp16 convert + per-partition sum(x)
                    nc.scalar.activation(
                        x16[:, sl], x_[:], AF.Copy,
                        accum_out=statA[:, j : j + 1],
                    )
                    # sum(x^2) from resident fp16 (keeps the staging ring
                    # recycling on the Copy alone; product -> dead pb)
                    nc.vector.scalar_tensor_tensor(
                        pb[:, sl], x16[:, sl], 1.0, x16[:, sl],
                        op0=ALU.mult, op1=ALU.mult,
                        accum_out=statA[:, NA + j : NA + j + 1],
                    )

                nc.vector.reduce_sum(stA[:, 0:1], statA[:, 0:NA], axis=AX.X)
                nc.vector.reduce_sum(stA[:, 1:2], statA[:, NA : 2 * NA], axis=AX.X)

            # ---- AG1: global input stats -> a, b ----
            nc.gpsimd.dma_start(cc_a_in[:], stA[:])
            all_gather(cc_a_in, cc_a_out)
            nc.gpsimd.dma_start(
                sAg[:].rearrange("p (k c) -> p k c", c=2),
                cc_a_out[:].rearrange("k p c -> p k c"),
            )
            nc.vector.reduce_sum(
                stA[:], sAg[:].rearrange("p (k c) -> p c k", c=2), axis=AX.X
            )
            nc.tensor.matmul(psumA[:], ones[:], stA[:])

            # var = S2/(N-1) - S1^2/(N(N-1)); a = scalei/sqrt(var); b = -S1/N*a
            nc.vector.tensor_copy(sm[:, 0:2], psumA[:])
            S1 = sm[:, 0:1]
            S2 = sm[:, 1:2]
            nc.vector.tensor_scalar(
                sm[:, 2:3], S1, S1, c_m2, op0=ALU.mult, op1=ALU.mult
            )
            nc.vector.tensor_scalar(
                sm[:, 3:4], S2, c_v, sm[:, 2:3], op0=ALU.mult, op1=ALU.add
            )
            nc.scalar.activation(
                sm[:, 2:3], sm[:, 3:4], AF.Abs_reciprocal_sqrt
            )                                                          # 1/std
            nc.vector.tensor_mul(sm[:, 4:5], sm[:, 2:3], scal_all[:, 0:1])  # a
            nc.vector.tensor_scalar(
                sm[:, 5:6], S1, sm[:, 4:5], c_mn, op0=ALU.mult, op1=ALU.mult
            )
            nc.vector.tensor_scalar_mul(sm[:, 6:8], sm[:, 4:6], 0.5)
            a_ap = sm[:, 4:5]
            b_ap = sm[:, 5:6]
            ah_ap = sm[:, 6:7]
            bh_ap = sm[:, 7:8]

            # dummy collective late in the mid phase keeps AG2 warm: the
            # dma depends on quarter-3 val stats (~75% through mid) so the
            # cc core is polling hot right before the real AG2 trigger
            # (after long idle its trigger->begin latency is ~8us)
            nc.gpsimd.dma_start(cc_w_in[:], statC[:, 12:14])
            all_gather(cc_w_in, cc_w_out)

            # ---------------- Mid ----------------
            with tc.tile_pool(name="pm", bufs=1) as pm:
                Tb = pm.tile([P, QMAX], F16, name="Tb", tag="Tb")

                QOFF = [sum(QWS[:i]) for i in range(NQ)]

                mm_count = [0]

                def emit_exp(q):
                    """exp batch for quarter q: e (in place), e', val, stats."""
                    qb = qbuf[q % 2]
                    qoff = QOFF[q]
                    for j0 in range(0, QWS[q], CC):
                        j = (qoff + j0) // CC
                        sl = slice(qoff + j0, qoff + j0 + CC)
                        lsl = slice(j0, j0 + CC)
                        nc.scalar.activation(
                            pb[:, sl], pb[:, sl], AF.Exp, scale=v_slope
                        )
                        # e' = u * e (4x fp16 tensor_tensor)
                        nc.vector.tensor_mul(pb[:, sl], x16[:, sl], pb[:, sl])
                        # val = v_slope*q + e' -> pb, accum sum(val)
                        nc.vector.scalar_tensor_tensor(
                            pb[:, sl], qb[:, lsl], v_slope, pb[:, sl],
                            op0=ALU.mult, op1=ALU.add,
                            accum_out=statC[:, j : j + 1],
                        )
                        # sum(val^2) on the idle PE: accumulate the gram
                        # diagonal of every [P,128] val tile into psumV
                        for k in range(CC // P):
                            t0 = qoff + j0 + k * P
                            tile_ap = pb[:, t0 : t0 + P]
                            idx = mm_count[0]
                            mm_count[0] += 1
                            nc.tensor.matmul(
                                psumV[:], tile_ap, tile_ap,
                                start=(idx == 0), stop=(idx == FREE // P - 1),
                            )

                for q in range(NQ):
                    w = QWS[q]
                    qoff = QOFF[q]
                    qsl = slice(qoff, qoff + w)
                    qb = qbuf[q % 2]
                    # t2 = tanh(d/2) -> pb
                    nc.scalar.activation(
                        pb[:, qsl], x16[:, qsl], AF.Tanh, bias=bh_ap, scale=ah_ap
                    )
                    # T = tanh(d) -> Tb (same tanh table, no reload)
                    nc.scalar.activation(
                        Tb[:, 0:w], x16[:, qsl], AF.Tanh, bias=b_ap, scale=a_ap
                    )
                    # u = a*x + b in place on the (now dead) x16 region;
                    # tensor_scalar runs at 2x fp16 vs the 1x custom AMR
                    for h0 in range(0, w, CT):
                        sl = slice(qoff + h0, qoff + h0 + CT)
                        nc.vector.tensor_scalar(
                            x16[:, sl], x16[:, sl], a_ap, b_ap,
                            op0=ALU.mult, op1=ALU.add,
                        )
                    # v_slope*|T| in place: clear the fp16 sign bit (2x
                    # tensor_scalar on the uint16 view), then scale by
                    # v_slope (2x) -- exp then runs with scale=1 and val
                    # becomes a pure tensor add
                    for h in range(2):
                        hsl = slice(h * (w // 2), (h + 1) * (w // 2))
                        tu = Tb[:, hsl].bitcast(mybir.dt.uint16)
                        nc.vector.tensor_single_scalar(
                            tu, tu, 0x7FFF, op=ALU.bitwise_and
                        )

                    # cos first (reads t2), then sin in place
                    nc.scalar.activation(
                        qb[:, 0:w], pb[:, qsl], AF.Sin,
                        bias=sm[:, 13:14], scale=th_half,
                    )
                    for h0 in range(0, w, CT):
                        hsl = slice(h0, h0 + CT)
                        nc.vector.tensor_mul(qb[:, hsl], qb[:, hsl], Tb[:, hsl])
                    nc.scalar.activation(
                        pb[:, qsl], pb[:, qsl], AF.Sin,
                        bias=sm[:, 12:13], scale=th_half,
                    )
                    for h0 in range(0, w, CT):
                        sl = slice(qoff + h0, qoff + h0 + CT)
                        nc.vector.tensor_mul(
                            pb[:, sl], pb[:, sl], Tb[:, h0 : h0 + CT]
                        )
                    emit_exp(q)

                nc.vector.reduce_sum(stB[:, 0:1], statC[:, 0:NC], axis=AX.X)
                # extract the gram diagonal: mask with identity, reduce
                nc.vector.tensor_mul(vdiag[:], psumV[:], iden[:])
                nc.vector.reduce_sum(stB[:, 1:2], vdiag[:], axis=AX.X)

            # ---- AG2: global val stats -> a2, b2 ----
            nc.gpsimd.dma_start(cc_b_in[:], stB[:])
            all_gather(cc_b_in, cc_b_out)
            nc.gpsimd.dma_start(
                sBg[:].rearrange("p (k c) -> p k c", c=2),
                cc_b_out[:].rearrange("k p c -> p k c"),
            )
            nc.vector.reduce_sum(
                stB[:], sBg[:].rearrange("p (k c) -> p c k", c=2), axis=AX.X
            )
            nc.tensor.matmul(psumB[:], ones[:], stB[:])

            nc.vector.tensor_copy(sm[:, 16:18], psumB[:])
            T1 = sm[:, 16:17]
            T2 = sm[:, 17:18]
            nc.vector.tensor_scalar(
                sm[:, 18:19], T1, T1, c_m2, op0=ALU.mult, op1=ALU.mult
            )
            nc.vector.tensor_scalar(
                sm[:, 19:20], T2, c_v, sm[:, 18:19], op0=ALU.mult, op1=ALU.add
            )
            nc.scalar.activation(
                sm[:, 18:19], sm[:, 19:20], AF.Abs_reciprocal_sqrt
            )
            nc.vector.tensor_mul(sm[:, 20:21], sm[:, 18:19], scal_all[:, 1:2])
            nc.vector.tensor_scalar(
                sm[:, 21:22], T1, sm[:, 20:21], c_mn, op0=ALU.mult, op1=ALU.mult
            )
            a2_ap = sm[:, 20:21]
            b2_ap = sm[:, 21:22]

            # ---------------- Phase D: normalize + store -------------
            with tc.tile_pool(name="pd", bufs=1) as pd:
                outs = [
                    pd.tile([P, CD], F32, name=f"o{i}", tag=f"o{i}")
                    for i in range(4)
                ]
                for j in range(ND):
                    sl = slice(j * CD, (j + 1) * CD)
                    o_ = outs[j % 4]
                    # out = a2*val + b2; alternate compute between the scalar
                    # and vector engines so neither serializes the stores
                    if j % 2 == 0:
                        nc.scalar.activation(
                            o_[:], pb[:, sl], AF.Identity, bias=b2_ap, scale=a2_ap
                        )
                        nc.sync.dma_start(out_dram[:, sl], o_[:])
                    else:
                        nc.vector.tensor_scalar(
                            o_[:], pb[:, sl], a2_ap, b2_ap,
                            op0=ALU.mult, op1=ALU.add,
                        )
                        nc.scalar.dma_start(out_dram[:, sl], o_[:])

    nc.finalize()
    return nc


def kernel(data, params, scalei, scaleo):
    global LAST_RESULT
    data = np.ascontiguousarray(np.asarray(data, dtype=np.float32))
    params = np.asarray(params, dtype=np.float32)

    # Affine-LUT coefficients from the actual params input.
    th_lut = params[0, 0]
    v_lut = params[1, 0]
    npts = th_lut.shape[0]
    th0 = float(th_lut[0])
    th_slope = float(th_lut[npts - 1]) - th0
    v0 = float(v_lut[0])
    v_slope = float(v_lut[npts - 1]) - v0
    assert abs(v0) < 1e-6, f"velocity LUT must start at 0 (got {v0})"

    # theta = th0 + th_slope*sigmoid(d) = th_mid + th_half*tanh(d/2)
    th_mid = th0 + 0.5 * th_slope
    th_half = 0.5 * th_slope

    consts = (th_mid, th_half, v_slope)
    nc = _KERNEL_CACHE.get(consts)
    if nc is None:
        nc = _build(consts)
        _KERNEL_CACHE[consts] = nc

    scal = np.tile(
        np.array(
            [[float(np.asarray(scalei).reshape(-1)[0]),
              float(np.asarray(scaleo).reshape(-1)[0])]],
            dtype=np.float32,
        ),
        (P, 1),
    )

    bpc = B_FULL // N_CORES
    in_maps = []
    for i in range(N_CORES):
        shard = np.ascontiguousarray(
            data[i * bpc : (i + 1) * bpc]
        ).reshape(P, FREE)
        in_maps.append({"data": shard, "scal": scal})

    res = run_bass_kernel_spmd(nc, in_maps, core_ids=list(range(N_CORES)))
    LAST_RESULT = res

    out = np.concatenate(
        [r["out"].reshape(bpc, C, H, W) for r in res.results], axis=0
    )
    return out
